# revision 60
# baseline (speedup 1.0000x reference)
"""LIF spiking-neuron recurrence kernel for Trainium2 (8 NeuronCores, SPMD).

Problem: x [32, 100, 8192] f32, decay [1] f32.
    d = sigmoid(decay)
    mem_0 = x[:,0];  mem_t = mem_{t-1} * d * (1 - spike_{t-1}) + x[:,t]
    spike_t = (mem_t > 0.5);  out[:,t] = spike_t  (f32 0/1)

Device formulation:
    W_{-1} = 0
    M_t = (W_{t-1} * d) + x_t
    W_t = (M_t <= 0.5) * M_t
spike_t = (M_t > 0.5) = (W_t == 0) exactly. The whole step is ONE custom
DVE op over a persistent self-referential W buffer (out AP trails in0 by
exactly F elements):
    LIF_STEP_ANT: out = M * (M <= s1),  M = in0*s0 + in1

v2 changes over the 61.9us baseline (measured ~38.9-41us on the 8-core
axon TRN2 pod, ambient-dependent; bit-identical to the exact host
simulation of the fp16 pipeline):
  * x is cast to fp16 on the host: input DMA bytes halve (13.1 -> 6.55 MB
    per core).  W state is fp16 too.  Exact-simulated vs the f32 jax
    reference: 1860/26.2M spike flips = rel err 1.802e-2 (< 2e-2 gate).
  * hand-written 2X_1PORT uop program for the LIF op (all operands fp16,
    stride-1, 4B-aligned): the serial recurrence runs 2 elem/cycle/lane,
    27.7us -> 14us measured.  The REGULAR 1x program stays in the table;
    HW falls back to it silently if the pattern check fails (the
    instruction's perf_max byte-36[7:6] requests the mode).
  * spike conversion emits a u8 notspike code, split DVE (tensor_scalar
    not_equal, {0,1}) / ACT (single Abs(1e19*W) pass, saturates to
    {0,255}); host decodes spike = (code == 0).
  * input loads on the Sync-engine HWDGE ring (fires ~1.5us earlier in
    the preamble than Scalar's), with DMA chunk 2 loaded over the Scalar
    ring in parallel; u8 stores drain on Scalar mid-run, Sync at the tail.

Sharding: 32*8192 = 262144 independent (b, d) lanes split 8 ways by
feature blocks; per-core layout [128 partitions, T*256] with partition
p = b*4 + (d_local//256), free offset = t*256 + d_local%256.
"""

from contextlib import ExitStack
from copy import deepcopy

import numpy as np

N_CORES = 8
B, T, D = 32, 100, 8192
P = 128          # SBUF partitions
F = 256          # free elements per timestep per core (32*1024/128)
THRESH = 0.5

# --- tuning knobs ---------------------------------------------------------
LIF_2X = True           # register + request the 2X_1PORT uop program
X_FP16 = True           # cast x to fp16 on host (halves input DMA)
# Spike conversion writes a u8 "notspike" code: DVE tensor_scalar is_not_equal
# gives (W!=0) in {0,1}; ACT in ONE pass writes u8(Abs(1e19*W)) which
# saturates to {0,255}.  Both decode on host as spike = (code == 0).
# (Shipping W itself (fp16) was tried: the doubled output DMA starved the
# input stream and lost 7us net.  GpSimd tensor_scalar is ~16x below
# roofline AND starves concurrent DVE SBUF access — never give it compute.)
DVE_FRAC = 0.35         # fraction of spike columns on DVE (rest on ACT)

_BUILD_CACHE: dict = {}
_LIF_OP = None

W_DT = "float16" if LIF_2X or X_FP16 else "float32"
X_DT = "float16" if X_FP16 else "float32"


def _chunk_schedule(t_steps: int):
    """(dma_chunks, lif_chunks, store_bounds).  DMA chunks are large so each
    SBUF row transfer is long (small fp16 rows gate per-packet DMA rate);
    LIF chunks stay small for pipelining; stores drain at ~25-step bounds
    with a tiny final store for a short tail."""
    if t_steps == 100:
        # Input is striped across BOTH HWDGE rings ("sync"/"scalar") so the
        # two queues stream in parallel: the LIF chain starts ~2us earlier
        # and the full input lands ~5us earlier.  Stores reuse the scalar
        # ring after its input chunks drain (~18us), sync at the tail.
        dma = [(0, 3, "sync"), (3, 3, "scalar"), (6, 10, "sync"),
               (16, 10, "scalar"), (26, 26, "sync"), (52, 32, "scalar"),
               (84, 16, "sync")]
        lif = [3, 3, 10, 10, 13, 13, 16, 16, 10, 6]
        bounds = [16, 39, 52, 68, 84, 94, 100]
        # tail < t_steps (ship raw W fp16, host-decoded) was tried: the
        # doubled tail-store bytes flushed slower than the spike passes
        # they replaced (44.3us vs 38.9us).
        return dma, lif, bounds, t_steps
    dma, rem, s = [], t_steps, 0
    while rem > 0:
        c = min(24, rem)
        dma.append((s, c, "sync"))
        s += c
        rem -= c
    bounds = [s_ + c_ for s_, c_, _ in dma]
    return dma, list(c for _, c, _ in dma), bounds, t_steps


def _build_2x_uop(u1x):
    """2X_1PORT program for LIF_STEP_ANT, built by editing a copy of the
    REGULAR program.  Element0 (LO) computes on ALU stages 0-3 exactly like
    the 1x program; element1 (HI) enters through SRC_*_HI input lanes, rides
    the delay lanes, and computes on stages 4-7.  The LO result is captured
    into delay lane 0 at stage 4 and written back from there."""
    from concourse.dve_uop import AluInp, AluOp, DelayInp, InpSel, OutPath, OutSel

    u = deepcopy(u1x)
    u.inp = [InpSel.ZERO, InpSel.SRC_0, InpSel.CONST_0, InpSel.SRC_1,
             InpSel.CONST_1, InpSel.SRC_0_HI, InpSel.SRC_1_HI, InpSel.ZERO]
    u.inp_enable = [0, 1, 1, 1, 1, 1, 1, 0]

    HOLD = DelayInp.PREV_DELAY
    CAP = DelayInp.PREV_ALU_OUT
    s = u.datapath_config
    # delay lanes at stage 0: d0=W0 d1=d d2=x0 d3=th d4=W1 d5=x1

    def dp(i, op, a, b, lane_caps=()):
        st = s[i]
        st.op = op
        st.alu_src0 = a
        st.alu_src1 = b
        st.alu_out_enable = 1
        st.delay = [CAP if k in lane_caps else HOLD for k in range(7)]
        st.delay_enable = [1, 1, 1, 1, 1, 1, 0]

    dp(0, AluOp.MULTIPLY, AluInp.PREV_DELAY_0, AluInp.PREV_DELAY_1)  # W0*d
    dp(1, AluOp.ADD, AluInp.PREV_ALU_OUT, AluInp.PREV_DELAY_2)       # M0
    dp(2, AluOp.IS_GE, AluInp.PREV_DELAY_3, AluInp.PREV_ALU_OUT,
       lane_caps=(0,))                                               # mask0; d0<-M0
    dp(3, AluOp.MULTIPLY, AluInp.PREV_DELAY_0, AluInp.PREV_ALU_OUT)  # out0
    dp(4, AluOp.MULTIPLY, AluInp.PREV_DELAY_4, AluInp.PREV_DELAY_1,
       lane_caps=(0,))                                               # W1*d; d0<-out0
    dp(5, AluOp.ADD, AluInp.PREV_ALU_OUT, AluInp.PREV_DELAY_5)       # M1
    dp(6, AluOp.IS_GE, AluInp.PREV_DELAY_3, AluInp.PREV_ALU_OUT,
       lane_caps=(1,))                                               # mask1; d1<-M1
    dp(7, AluOp.MULTIPLY, AluInp.PREV_DELAY_1, AluInp.PREV_ALU_OUT)  # out1

    u.out = {OutPath.WR0_LO: OutSel.DELAY_0, OutPath.WR0_HI: OutSel.ALU_OUT,
             OutPath.WR1_LO: OutSel.ALU_OUT, OutPath.WR1_HI: OutSel.ALU_OUT}
    u.out_enable = {OutPath.WR0_LO: 1, OutPath.WR0_HI: 1,
                    OutPath.WR1_LO: 0, OutPath.WR1_HI: 0}
    return u


def _get_lif_op():
    """Register the fused LIF-step custom DVE op (idempotent)."""
    global _LIF_OP
    if _LIF_OP is not None:
        return _LIF_OP
    from concourse.dve_ops import (
        _COMPILE_CACHE, CUSTOM_DVE_SPECS, OPS, _SUB_OPCODE_FOR_NAME, DveOp,
    )
    from concourse.dve_spec import C0, C1, Spec, Src0, Src1, lower
    from concourse.dve_table_gen import dve_ver_for
    from concourse.dve_uop import DveOpSpec

    name = "LIF_STEP_ANT"
    if name in _SUB_OPCODE_FOR_NAME:
        _LIF_OP = next(op for op in OPS if op.name == name)
        return _LIF_OP

    M = Src0 * C0 + Src1

    def _ref(in0, in1, s0, s1, imm2):
        m = (in0.astype(np.float32) * np.float32(s0)
             + in1.astype(np.float32)).astype(np.float32)
        return np.where(m <= np.float32(s1), m, np.float32(0.0)).astype(np.float32)

    spec = Spec(body=M * (M <= C1), reference=_ref)
    row = max(_SUB_OPCODE_FOR_NAME.values()) + 1
    assert row < 0x20
    _SUB_OPCODE_FOR_NAME[name] = row

    assert dve_ver_for("TRN2") == "v3"
    uops_1x = lower(spec, ver="v3")
    assert len(uops_1x) == 1
    opspec = DveOpSpec(
        name=name, opcode=row, uops=uops_1x,
        uops_2x=[_build_2x_uop(uops_1x[0])] if LIF_2X else None,
        rd1_en=True, perf_max=1 if LIF_2X else 0,
    )
    shas = {"v3": opspec.sha("v3")}
    op = DveOp(name, spec, subdim=False, uops_sha=shas)
    OPS.append(op)
    CUSTOM_DVE_SPECS[name] = spec
    _COMPILE_CACHE[(name, "v3")] = opspec  # bypass lower() at table-gen/emit
    _LIF_OP = op
    return op


def _build_nc(t_steps: int, d_imm: float):
    import concourse.tile as tile
    from concourse import bacc, mybir

    w_dt = getattr(mybir.dt, W_DT)
    x_dt = getattr(mybir.dt, X_DT)

    lif_op = _get_lif_op()
    dma_chunks, lif_chunks, store_bounds, tail = _chunk_schedule(t_steps)
    assert sum(c for _, c, _ in dma_chunks) == t_steps
    assert sum(lif_chunks) == t_steps
    assert not store_bounds or store_bounds[-1] <= tail
    max_dc = max(c for _, c, _ in dma_chunks)

    nc = bacc.Bacc("TRN2", debug=False, target_bir_lowering=False)
    x_in = nc.dram_tensor("x", [P, t_steps * F], x_dt, kind="ExternalInput")
    s_out = nc.dram_tensor("s", [P, max(tail, 1) * F], mybir.dt.uint8,
                           kind="ExternalOutput")
    # steps >= tail ship as raw W (fp16); host decodes spike = (W == 0)
    w_out = (nc.dram_tensor("w", [P, (t_steps - tail) * F], w_dt,
                            kind="ExternalOutput")
             if tail < t_steps else None)

    with tile.TileContext(nc) as tcx, ExitStack() as ctx:
        xpool = ctx.enter_context(tcx.tile_pool(name="xp", bufs=4))
        spool = ctx.enter_context(tcx.tile_pool(name="sp", bufs=1))

        # Persistent state buffer: W[:, t*F:(t+1)*F] holds W_{t-1} (slot 0 is
        # the zero initial state, slot t+1 is W_t).
        wbuf = spool.tile([P, (t_steps + 1) * F], w_dt)
        nc.vector.memset(wbuf[:, 0:F], 0.0)
        # Persistent notspike-code buffer (u8; 0 <=> spike) for steps < tail.
        stbuf = spool.tile([P, max(tail, 1) * F], mybir.dt.uint8)

        def emit_spike(t0_, tc, mode="split"):
            # mode: "split" DVE/ACT by DVE_FRAC; "act" all-ACT (tail chunks,
            # overlaps the still-running LIF chain); "dve" all-DVE (the very
            # last chunk — shortest serial tail once the LIF chain is done).
            n = tc * F
            c_dve = {"split": (int(n * DVE_FRAC) // 16) * 16,
                     "act": 0, "dve": n}[mode]
            wsl = wbuf[:, (t0_ + 1) * F:(t0_ + tc + 1) * F]
            st = stbuf[:, t0_ * F:(t0_ + tc) * F]
            if c_dve > 0:
                nc.vector.tensor_scalar(
                    out=st[:, :c_dve], in0=wsl[:, :c_dve],
                    scalar1=0.0, scalar2=None,
                    op0=mybir.AluOpType.not_equal)
            if c_dve < n:
                nc.scalar.activation(
                    out=st[:, c_dve:n], in_=wsl[:, c_dve:n],
                    func=mybir.ActivationFunctionType.Abs, scale=1e19)

        def emit_store(s0_, s1_, eng):
            eng.dma_start(out=s_out[:, s0_ * F:s1_ * F],
                          in_=stbuf[:, s0_ * F:s1_ * F])

        # Input DMA chunks are wide (long SBUF rows sustain queue rate); LIF
        # consumes each loaded tile in smaller sub-chunks.  Spike passes trail
        # the LIF by one chunk; stores drain at store_bounds on the Scalar
        # ring, the last ones on the (by then input-free) Sync ring.
        dma_by_start = {s_: (c_, e_) for s_, c_, e_ in dma_chunks}
        xtiles = []  # (t0, dc, xt) of loaded chunks

        t0 = 0
        stored = 0
        bounds = list(store_bounds)
        prev = None
        pend = None  # contiguous [start, end) LIF-done but not yet spiked

        def flush_spikes(done, mode="split"):
            nonlocal pend, stored
            if pend is not None:
                emit_spike(pend[0], pend[1] - pend[0], mode=mode)
                pend = None
            while bounds and done >= bounds[0]:
                eng = nc.scalar if bounds[0] < 84 else nc.sync
                emit_store(stored, bounds[0], eng)
                stored = bounds.pop(0)

        for k, tc in enumerate(lif_chunks):
            if t0 in dma_by_start:
                dc, ename = dma_by_start[t0]
                xt = xpool.tile([P, max_dc * F], x_dt, tag="xt")
                eng = nc.sync if ename == "sync" else nc.scalar
                eng.dma_start(out=xt[:, :dc * F],
                              in_=x_in[:, t0 * F:(t0 + dc) * F])
                xtiles.append((t0, dc, xt))
            xt0, _, xt = next(e for e in xtiles if e[0] <= t0 < e[0] + e[1])
            xoff = t0 - xt0
            ins = nc.vector._custom_dve(
                lif_op,
                out=wbuf[:, (t0 + 1) * F:(t0 + tc + 1) * F],
                in0=wbuf[:, t0 * F:(t0 + tc) * F],
                in1=xt[:, xoff * F:(xoff + tc) * F],
                s0=d_imm, s1=THRESH)
            if LIF_2X:
                ins.ins.perf_max = 1  # byte-36[7:6] -> engine runs 2X_1PORT
            if prev is not None and prev[0] < tail:
                done = prev[0] + prev[1]
                if prev[0] >= 84:
                    # tail chunks: per-chunk, all-DVE (shortest serial tail)
                    emit_spike(*prev, mode="dve")
                    while bounds and done >= bounds[0]:
                        emit_store(stored, bounds[0], nc.sync)
                        stored = bounds.pop(0)
                else:
                    # Per-chunk spike slices.  (Merging slices across chunks
                    # to amortize instruction overhead was tried at several
                    # granularities: the long ACT slices delay store triggers
                    # queued behind them and always measured ~1-3us worse.)
                    pend = (prev[0] if pend is None else pend[0], done)
                    flush_spikes(done)
            prev = (t0, tc)
            t0 += tc
        if prev[0] < tail:
            flush_spikes(prev[0])
            emit_spike(*prev, mode="dve")
        for b in bounds:
            emit_store(stored, b, nc.sync)
            stored = b
        if w_out is not None:
            # Tail W stores: split across both rings so they flush in
            # parallel right after the last LIF chunk.
            mid = tail + (t_steps - tail) // 2
            nc.sync.dma_start(out=w_out[:, 0:(mid - tail) * F],
                              in_=wbuf[:, (tail + 1) * F:(mid + 1) * F])
            nc.scalar.dma_start(
                out=w_out[:, (mid - tail) * F:(t_steps - tail) * F],
                in_=wbuf[:, (mid + 1) * F:(t_steps + 1) * F])
    nc.compile()
    return nc


def _get_nc(t_steps: int, d_imm: float):
    key = (t_steps, np.float32(d_imm).tobytes(), LIF_2X, X_DT, W_DT)
    if key not in _BUILD_CACHE:
        _BUILD_CACHE[key] = _build_nc(t_steps, d_imm)
    return _BUILD_CACHE[key]


def _shard_x(x: np.ndarray) -> list[np.ndarray]:
    b, t, d = x.shape
    # [b, t, core, chunk, 256] -> [core, b, chunk, t, 256] -> [core, 128, t*256]
    xr = x.reshape(b, t, N_CORES, 4, F).transpose(2, 0, 3, 1, 4)
    xr = np.ascontiguousarray(xr).reshape(N_CORES, P, t * F)
    if X_FP16:
        xr = xr.astype(np.float16)
    return [xr[c] for c in range(N_CORES)]


def _unshard_spikes(codes: np.ndarray, t: int) -> np.ndarray:
    # codes: [core, 128, t*256] with 0 <=> spike (u8 notspike code for the
    # head steps, raw W fp16 bits for the tail steps — both decode the same).
    sr = (codes == 0).astype(np.float32).reshape(N_CORES, B, 4, t, F)
    sr = sr.transpose(1, 3, 0, 2, 4)
    return np.ascontiguousarray(sr).reshape(B, t, N_CORES * 4 * F)


def _sigmoid_f32(decay: np.ndarray) -> np.float32:
    import jax
    import jax.numpy as jnp
    d = np.asarray(jax.nn.sigmoid(jnp.asarray(decay, jnp.float32)))
    return np.float32(d.reshape(-1)[0])


def kernel(x: np.ndarray, decay: np.ndarray) -> np.ndarray:
    from concourse.bass_utils import run_bass_kernel_spmd

    x = np.asarray(x, dtype=np.float32)
    b, t, d = x.shape
    d_f32 = _sigmoid_f32(np.asarray(decay))

    nc = _get_nc(t, float(d_f32))
    shards = _shard_x(x)
    in_maps = [{"x": np.ascontiguousarray(s)} for s in shards]
    res = run_bass_kernel_spmd(nc, in_maps, core_ids=list(range(N_CORES)))
    parts = []
    for c in range(N_CORES):
        r = res.results[c]
        code = np.asarray(r["s"]) != 0
        if "w" in r:
            code = np.concatenate([code, np.asarray(r["w"]) != 0], axis=1)
        parts.append(code)
    return _unshard_spikes(np.stack(parts, axis=0), t)


# revision 61
# speedup vs baseline: 1.0532x; 1.0532x over previous
"""LIF spiking-neuron recurrence kernel for Trainium2 (8 NeuronCores, SPMD).

Problem: x [32, 100, 8192] f32, decay [1] f32.
    d = sigmoid(decay)
    mem_0 = x[:,0];  mem_t = mem_{t-1} * d * (1 - spike_{t-1}) + x[:,t]
    spike_t = (mem_t > 0.5);  out[:,t] = spike_t  (f32 0/1)

Device formulation:
    W_{-1} = 0
    M_t = (W_{t-1} * d) + x_t
    W_t = (M_t <= 0.5) * M_t
spike_t = (M_t > 0.5) = (W_t == 0) exactly. The whole step is ONE custom
DVE op over a persistent self-referential W buffer (out AP trails in0 by
exactly F elements):
    LIF_STEP_ANT: out = M * (M <= s1),  M = in0*s0 + in1

v2 changes over the 61.9us baseline (measured ~38.9-41us on the 8-core
axon TRN2 pod, ambient-dependent; bit-identical to the exact host
simulation of the fp16 pipeline):
  * x is cast to fp16 on the host: input DMA bytes halve (13.1 -> 6.55 MB
    per core).  W state is fp16 too.  Exact-simulated vs the f32 jax
    reference: 1860/26.2M spike flips = rel err 1.802e-2 (< 2e-2 gate).
  * hand-written 2X_1PORT uop program for the LIF op (all operands fp16,
    stride-1, 4B-aligned): the serial recurrence runs 2 elem/cycle/lane,
    27.7us -> 14us measured.  The REGULAR 1x program stays in the table;
    HW falls back to it silently if the pattern check fails (the
    instruction's perf_max byte-36[7:6] requests the mode).
  * spike conversion emits a u8 notspike code, split DVE (tensor_scalar
    not_equal, {0,1}) / ACT (single Abs(1e19*W) pass, saturates to
    {0,255}); host decodes spike = (code == 0).
  * input loads on the Sync-engine HWDGE ring (fires ~1.5us earlier in
    the preamble than Scalar's), with DMA chunk 2 loaded over the Scalar
    ring in parallel; u8 stores drain on Scalar mid-run, Sync at the tail.

Sharding: 32*8192 = 262144 independent (b, d) lanes split 8 ways by
feature blocks; per-core layout [128 partitions, T*256] with partition
p = b*4 + (d_local//256), free offset = t*256 + d_local%256.
"""

from contextlib import ExitStack
from copy import deepcopy

import numpy as np

N_CORES = 8
B, T, D = 32, 100, 8192
P = 128          # SBUF partitions
F = 256          # free elements per timestep per core (32*1024/128)
THRESH = 0.5

# --- tuning knobs ---------------------------------------------------------
LIF_2X = True           # register + request the 2X_1PORT uop program
X_FP16 = True           # cast x to fp16 on host (halves input DMA)
# Spike conversion writes a u8 "notspike" code: DVE tensor_scalar is_not_equal
# gives (W!=0) in {0,1}; ACT in ONE pass writes u8(Abs(1e19*W)) which
# saturates to {0,255}.  Both decode on host as spike = (code == 0).
# (Shipping W itself (fp16) was tried: the doubled output DMA starved the
# input stream and lost 7us net.  GpSimd tensor_scalar is ~16x below
# roofline AND starves concurrent DVE SBUF access — never give it compute.)
DVE_FRAC = 0.35         # fraction of spike columns on DVE (rest on ACT)

_BUILD_CACHE: dict = {}
_LIF_OP = None

W_DT = "float16" if LIF_2X or X_FP16 else "float32"
X_DT = "float16" if X_FP16 else "float32"


def _chunk_schedule(t_steps: int):
    """(dma_chunks, lif_chunks, store_bounds).  DMA chunks are large so each
    SBUF row transfer is long (small fp16 rows gate per-packet DMA rate);
    LIF chunks stay small for pipelining; stores drain at ~25-step bounds
    with a tiny final store for a short tail."""
    if t_steps == 100:
        import os
        if os.environ.get("LIF_SCHED") == "A":
            # checkpoint schedule: input on sync only (chunk 2 on scalar)
            dma = [(0, 6, "sync"), (6, 20, "scalar"), (26, 26, "sync"),
                   (52, 32, "sync"), (84, 16, "sync")]
            lif = [6, 10, 10, 13, 13, 16, 16, 10, 6]
            bounds = [16, 39, 52, 68, 84, 94, 100]
            return dma, lif, bounds, t_steps
        # Input is striped across BOTH HWDGE rings ("sync"/"scalar") so the
        # two queues stream in parallel: the LIF chain starts ~2us earlier
        # and the full input lands ~5us earlier.  Stores reuse the scalar
        # ring after its input chunks drain (~18us), sync at the tail.
        dma = [(0, 3, "sync"), (3, 3, "scalar"), (6, 10, "sync"),
               (16, 10, "scalar"), (26, 26, "sync"), (52, 32, "scalar"),
               (84, 16, "sync")]
        lif = [3, 3, 10, 10, 13, 13, 16, 16, 10, 6]
        bounds = [16, 39, 52, 68, 84, 94, 100]
        # tail < t_steps (ship raw W fp16, host-decoded) was tried: the
        # doubled tail-store bytes flushed slower than the spike passes
        # they replaced (44.3us vs 38.9us).
        return dma, lif, bounds, t_steps
    dma, rem, s = [], t_steps, 0
    while rem > 0:
        c = min(24, rem)
        dma.append((s, c, "sync"))
        s += c
        rem -= c
    bounds = [s_ + c_ for s_, c_, _ in dma]
    return dma, list(c for _, c, _ in dma), bounds, t_steps


def _build_2x_uop(u1x):
    """2X_1PORT program for LIF_STEP_ANT, built by editing a copy of the
    REGULAR program.  Element0 (LO) computes on ALU stages 0-3 exactly like
    the 1x program; element1 (HI) enters through SRC_*_HI input lanes, rides
    the delay lanes, and computes on stages 4-7.  The LO result is captured
    into delay lane 0 at stage 4 and written back from there."""
    from concourse.dve_uop import AluInp, AluOp, DelayInp, InpSel, OutPath, OutSel

    u = deepcopy(u1x)
    u.inp = [InpSel.ZERO, InpSel.SRC_0, InpSel.CONST_0, InpSel.SRC_1,
             InpSel.CONST_1, InpSel.SRC_0_HI, InpSel.SRC_1_HI, InpSel.ZERO]
    u.inp_enable = [0, 1, 1, 1, 1, 1, 1, 0]

    HOLD = DelayInp.PREV_DELAY
    CAP = DelayInp.PREV_ALU_OUT
    s = u.datapath_config
    # delay lanes at stage 0: d0=W0 d1=d d2=x0 d3=th d4=W1 d5=x1

    def dp(i, op, a, b, lane_caps=()):
        st = s[i]
        st.op = op
        st.alu_src0 = a
        st.alu_src1 = b
        st.alu_out_enable = 1
        st.delay = [CAP if k in lane_caps else HOLD for k in range(7)]
        st.delay_enable = [1, 1, 1, 1, 1, 1, 0]

    dp(0, AluOp.MULTIPLY, AluInp.PREV_DELAY_0, AluInp.PREV_DELAY_1)  # W0*d
    dp(1, AluOp.ADD, AluInp.PREV_ALU_OUT, AluInp.PREV_DELAY_2)       # M0
    dp(2, AluOp.IS_GE, AluInp.PREV_DELAY_3, AluInp.PREV_ALU_OUT,
       lane_caps=(0,))                                               # mask0; d0<-M0
    dp(3, AluOp.MULTIPLY, AluInp.PREV_DELAY_0, AluInp.PREV_ALU_OUT)  # out0
    dp(4, AluOp.MULTIPLY, AluInp.PREV_DELAY_4, AluInp.PREV_DELAY_1,
       lane_caps=(0,))                                               # W1*d; d0<-out0
    dp(5, AluOp.ADD, AluInp.PREV_ALU_OUT, AluInp.PREV_DELAY_5)       # M1
    dp(6, AluOp.IS_GE, AluInp.PREV_DELAY_3, AluInp.PREV_ALU_OUT,
       lane_caps=(1,))                                               # mask1; d1<-M1
    dp(7, AluOp.MULTIPLY, AluInp.PREV_DELAY_1, AluInp.PREV_ALU_OUT)  # out1

    u.out = {OutPath.WR0_LO: OutSel.DELAY_0, OutPath.WR0_HI: OutSel.ALU_OUT,
             OutPath.WR1_LO: OutSel.ALU_OUT, OutPath.WR1_HI: OutSel.ALU_OUT}
    u.out_enable = {OutPath.WR0_LO: 1, OutPath.WR0_HI: 1,
                    OutPath.WR1_LO: 0, OutPath.WR1_HI: 0}
    return u


def _get_lif_op():
    """Register the fused LIF-step custom DVE op (idempotent)."""
    global _LIF_OP
    if _LIF_OP is not None:
        return _LIF_OP
    from concourse.dve_ops import (
        _COMPILE_CACHE, CUSTOM_DVE_SPECS, OPS, _SUB_OPCODE_FOR_NAME, DveOp,
    )
    from concourse.dve_spec import C0, C1, Spec, Src0, Src1, lower
    from concourse.dve_table_gen import dve_ver_for
    from concourse.dve_uop import DveOpSpec

    name = "LIF_STEP_ANT"
    if name in _SUB_OPCODE_FOR_NAME:
        _LIF_OP = next(op for op in OPS if op.name == name)
        return _LIF_OP

    M = Src0 * C0 + Src1

    def _ref(in0, in1, s0, s1, imm2):
        m = (in0.astype(np.float32) * np.float32(s0)
             + in1.astype(np.float32)).astype(np.float32)
        return np.where(m <= np.float32(s1), m, np.float32(0.0)).astype(np.float32)

    spec = Spec(body=M * (M <= C1), reference=_ref)
    row = max(_SUB_OPCODE_FOR_NAME.values()) + 1
    assert row < 0x20
    _SUB_OPCODE_FOR_NAME[name] = row

    assert dve_ver_for("TRN2") == "v3"
    uops_1x = lower(spec, ver="v3")
    assert len(uops_1x) == 1
    opspec = DveOpSpec(
        name=name, opcode=row, uops=uops_1x,
        uops_2x=[_build_2x_uop(uops_1x[0])] if LIF_2X else None,
        rd1_en=True, perf_max=1 if LIF_2X else 0,
    )
    shas = {"v3": opspec.sha("v3")}
    op = DveOp(name, spec, subdim=False, uops_sha=shas)
    OPS.append(op)
    CUSTOM_DVE_SPECS[name] = spec
    _COMPILE_CACHE[(name, "v3")] = opspec  # bypass lower() at table-gen/emit
    _LIF_OP = op
    return op


def _build_nc(t_steps: int, d_imm: float):
    import concourse.tile as tile
    from concourse import bacc, mybir

    w_dt = getattr(mybir.dt, W_DT)
    x_dt = getattr(mybir.dt, X_DT)

    lif_op = _get_lif_op()
    dma_chunks, lif_chunks, store_bounds, tail = _chunk_schedule(t_steps)
    assert sum(c for _, c, _ in dma_chunks) == t_steps
    assert sum(lif_chunks) == t_steps
    assert not store_bounds or store_bounds[-1] <= tail
    max_dc = max(c for _, c, _ in dma_chunks)

    nc = bacc.Bacc("TRN2", debug=False, target_bir_lowering=False)
    x_in = nc.dram_tensor("x", [P, t_steps * F], x_dt, kind="ExternalInput")
    s_out = nc.dram_tensor("s", [P, max(tail, 1) * F], mybir.dt.uint8,
                           kind="ExternalOutput")
    # steps >= tail ship as raw W (fp16); host decodes spike = (W == 0)
    w_out = (nc.dram_tensor("w", [P, (t_steps - tail) * F], w_dt,
                            kind="ExternalOutput")
             if tail < t_steps else None)

    with tile.TileContext(nc) as tcx, ExitStack() as ctx:
        xpool = ctx.enter_context(tcx.tile_pool(name="xp", bufs=4))
        spool = ctx.enter_context(tcx.tile_pool(name="sp", bufs=1))

        # Persistent state buffer: W[:, t*F:(t+1)*F] holds W_{t-1} (slot 0 is
        # the zero initial state, slot t+1 is W_t).
        wbuf = spool.tile([P, (t_steps + 1) * F], w_dt)
        nc.vector.memset(wbuf[:, 0:F], 0.0)
        # Persistent notspike-code buffer (u8; 0 <=> spike) for steps < tail.
        stbuf = spool.tile([P, max(tail, 1) * F], mybir.dt.uint8)

        def emit_spike(t0_, tc, mode="split"):
            # mode: "split" DVE/ACT by DVE_FRAC; "act" all-ACT (tail chunks,
            # overlaps the still-running LIF chain); "dve" all-DVE (the very
            # last chunk — shortest serial tail once the LIF chain is done).
            n = tc * F
            c_dve = {"split": (int(n * DVE_FRAC) // 16) * 16,
                     "act": 0, "dve": n}[mode]
            wsl = wbuf[:, (t0_ + 1) * F:(t0_ + tc + 1) * F]
            st = stbuf[:, t0_ * F:(t0_ + tc) * F]
            if c_dve > 0:
                nc.vector.tensor_scalar(
                    out=st[:, :c_dve], in0=wsl[:, :c_dve],
                    scalar1=0.0, scalar2=None,
                    op0=mybir.AluOpType.not_equal)
            if c_dve < n:
                nc.scalar.activation(
                    out=st[:, c_dve:n], in_=wsl[:, c_dve:n],
                    func=mybir.ActivationFunctionType.Abs, scale=1e19)

        def emit_store(s0_, s1_, eng):
            eng.dma_start(out=s_out[:, s0_ * F:s1_ * F],
                          in_=stbuf[:, s0_ * F:s1_ * F])

        # Input DMA chunks are wide (long SBUF rows sustain queue rate); LIF
        # consumes each loaded tile in smaller sub-chunks.  Spike passes trail
        # the LIF by one chunk; stores drain at store_bounds on the Scalar
        # ring, the last ones on the (by then input-free) Sync ring.
        dma_by_start = {s_: (c_, e_) for s_, c_, e_ in dma_chunks}
        xtiles = []  # (t0, dc, xt) of loaded chunks

        t0 = 0
        stored = 0
        bounds = list(store_bounds)
        prev = None
        pend = None  # contiguous [start, end) LIF-done but not yet spiked

        def flush_spikes(done, mode="split"):
            nonlocal pend, stored
            if pend is not None:
                emit_spike(pend[0], pend[1] - pend[0], mode=mode)
                pend = None
            while bounds and done >= bounds[0]:
                eng = nc.scalar if bounds[0] < 84 else nc.sync
                emit_store(stored, bounds[0], eng)
                stored = bounds.pop(0)

        for k, tc in enumerate(lif_chunks):
            if t0 in dma_by_start:
                dc, ename = dma_by_start[t0]
                xt = xpool.tile([P, max_dc * F], x_dt, tag="xt")
                eng = nc.sync if ename == "sync" else nc.scalar
                eng.dma_start(out=xt[:, :dc * F],
                              in_=x_in[:, t0 * F:(t0 + dc) * F])
                xtiles.append((t0, dc, xt))
            xt0, _, xt = next(e for e in xtiles if e[0] <= t0 < e[0] + e[1])
            xoff = t0 - xt0
            ins = nc.vector._custom_dve(
                lif_op,
                out=wbuf[:, (t0 + 1) * F:(t0 + tc + 1) * F],
                in0=wbuf[:, t0 * F:(t0 + tc) * F],
                in1=xt[:, xoff * F:(xoff + tc) * F],
                s0=d_imm, s1=THRESH)
            if LIF_2X:
                ins.ins.perf_max = 1  # byte-36[7:6] -> engine runs 2X_1PORT
            if prev is not None and prev[0] < tail:
                done = prev[0] + prev[1]
                if prev[0] >= 84:
                    # tail chunks: per-chunk, all-DVE (shortest serial tail)
                    emit_spike(*prev, mode="dve")
                    while bounds and done >= bounds[0]:
                        emit_store(stored, bounds[0], nc.sync)
                        stored = bounds.pop(0)
                else:
                    # Per-chunk spike slices.  (Merging slices across chunks
                    # to amortize instruction overhead was tried at several
                    # granularities: the long ACT slices delay store triggers
                    # queued behind them and always measured ~1-3us worse.)
                    pend = (prev[0] if pend is None else pend[0], done)
                    flush_spikes(done)
            prev = (t0, tc)
            t0 += tc
        if prev[0] < tail:
            flush_spikes(prev[0])
            emit_spike(*prev, mode="dve")
        for b in bounds:
            emit_store(stored, b, nc.sync)
            stored = b
        if w_out is not None:
            # Tail W stores: split across both rings so they flush in
            # parallel right after the last LIF chunk.
            mid = tail + (t_steps - tail) // 2
            nc.sync.dma_start(out=w_out[:, 0:(mid - tail) * F],
                              in_=wbuf[:, (tail + 1) * F:(mid + 1) * F])
            nc.scalar.dma_start(
                out=w_out[:, (mid - tail) * F:(t_steps - tail) * F],
                in_=wbuf[:, (mid + 1) * F:(t_steps + 1) * F])
    nc.compile()
    return nc


def _get_nc(t_steps: int, d_imm: float):
    key = (t_steps, np.float32(d_imm).tobytes(), LIF_2X, X_DT, W_DT)
    if key not in _BUILD_CACHE:
        _BUILD_CACHE[key] = _build_nc(t_steps, d_imm)
    return _BUILD_CACHE[key]


def _shard_x(x: np.ndarray) -> list[np.ndarray]:
    b, t, d = x.shape
    # [b, t, core, chunk, 256] -> [core, b, chunk, t, 256] -> [core, 128, t*256]
    xr = x.reshape(b, t, N_CORES, 4, F).transpose(2, 0, 3, 1, 4)
    xr = np.ascontiguousarray(xr).reshape(N_CORES, P, t * F)
    if X_FP16:
        xr = xr.astype(np.float16)
    return [xr[c] for c in range(N_CORES)]


def _unshard_spikes(codes: np.ndarray, t: int) -> np.ndarray:
    # codes: [core, 128, t*256] with 0 <=> spike (u8 notspike code for the
    # head steps, raw W fp16 bits for the tail steps — both decode the same).
    sr = (codes == 0).astype(np.float32).reshape(N_CORES, B, 4, t, F)
    sr = sr.transpose(1, 3, 0, 2, 4)
    return np.ascontiguousarray(sr).reshape(B, t, N_CORES * 4 * F)


def _sigmoid_f32(decay: np.ndarray) -> np.float32:
    import jax
    import jax.numpy as jnp
    d = np.asarray(jax.nn.sigmoid(jnp.asarray(decay, jnp.float32)))
    return np.float32(d.reshape(-1)[0])


def kernel(x: np.ndarray, decay: np.ndarray) -> np.ndarray:
    from concourse.bass_utils import run_bass_kernel_spmd

    x = np.asarray(x, dtype=np.float32)
    b, t, d = x.shape
    d_f32 = _sigmoid_f32(np.asarray(decay))

    nc = _get_nc(t, float(d_f32))
    shards = _shard_x(x)
    in_maps = [{"x": np.ascontiguousarray(s)} for s in shards]
    res = run_bass_kernel_spmd(nc, in_maps, core_ids=list(range(N_CORES)))
    parts = []
    for c in range(N_CORES):
        r = res.results[c]
        code = np.asarray(r["s"]) != 0
        if "w" in r:
            code = np.concatenate([code, np.asarray(r["w"]) != 0], axis=1)
        parts.append(code)
    return _unshard_spikes(np.stack(parts, axis=0), t)


# revision 62
# speedup vs baseline: 1.1679x; 1.1089x over previous
"""LIF spiking-neuron recurrence kernel for Trainium2 (8 NeuronCores, SPMD).

Problem: x [32, 100, 8192] f32, decay [1] f32.
    d = sigmoid(decay)
    mem_0 = x[:,0];  mem_t = mem_{t-1} * d * (1 - spike_{t-1}) + x[:,t]
    spike_t = (mem_t > 0.5);  out[:,t] = spike_t  (f32 0/1)

Device formulation:
    W_{-1} = 0
    M_t = (W_{t-1} * d) + x_t
    W_t = (M_t <= 0.5) * M_t
spike_t = (M_t > 0.5) = (W_t == 0) exactly. The whole step is ONE custom
DVE op over a persistent self-referential W buffer (out AP trails in0 by
exactly F elements):
    LIF_STEP_ANT: out = M * (M <= s1),  M = in0*s0 + in1

v2 changes over the 61.9us baseline (measured ~38.9-41us on the 8-core
axon TRN2 pod, ambient-dependent; bit-identical to the exact host
simulation of the fp16 pipeline):
  * x is cast to fp16 on the host: input DMA bytes halve (13.1 -> 6.55 MB
    per core).  W state is fp16 too.  Exact-simulated vs the f32 jax
    reference: 1860/26.2M spike flips = rel err 1.802e-2 (< 2e-2 gate).
  * hand-written 2X_1PORT uop program for the LIF op (all operands fp16,
    stride-1, 4B-aligned): the serial recurrence runs 2 elem/cycle/lane,
    27.7us -> 14us measured.  The REGULAR 1x program stays in the table;
    HW falls back to it silently if the pattern check fails (the
    instruction's perf_max byte-36[7:6] requests the mode).
  * spike conversion emits a u8 notspike code, split DVE (tensor_scalar
    not_equal, {0,1}) / ACT (single Abs(1e19*W) pass, saturates to
    {0,255}); host decodes spike = (code == 0).
  * input loads on the Sync-engine HWDGE ring (fires ~1.5us earlier in
    the preamble than Scalar's), with DMA chunk 2 loaded over the Scalar
    ring in parallel; u8 stores drain on Scalar mid-run, Sync at the tail.

Sharding: 32*8192 = 262144 independent (b, d) lanes split 8 ways by
feature blocks; per-core layout [128 partitions, T*256] with partition
p = b*4 + (d_local//256), free offset = t*256 + d_local%256.
"""

from contextlib import ExitStack
from copy import deepcopy

import numpy as np

N_CORES = 8
B, T, D = 32, 100, 8192
P = 128          # SBUF partitions
F = 256          # free elements per timestep per core (32*1024/128)
THRESH = 0.5

# --- tuning knobs ---------------------------------------------------------
LIF_2X = True           # register + request the 2X_1PORT uop program
X_FP16 = True           # cast x to fp16 on host (halves input DMA)
# Spike conversion writes a u8 "notspike" code: DVE tensor_scalar is_not_equal
# gives (W!=0) in {0,1}; ACT in ONE pass writes u8(Abs(1e19*W)) which
# saturates to {0,255}.  Both decode on host as spike = (code == 0).
# (Shipping W itself (fp16) was tried: the doubled output DMA starved the
# input stream and lost 7us net.  GpSimd tensor_scalar is ~16x below
# roofline AND starves concurrent DVE SBUF access — never give it compute.)
DVE_FRAC = 0.35         # fraction of spike columns on DVE (rest on ACT)

_BUILD_CACHE: dict = {}
_LIF_OP = None

W_DT = "float16" if LIF_2X or X_FP16 else "float32"
X_DT = "float16" if X_FP16 else "float32"


def _chunk_schedule(t_steps: int):
    """(dma_chunks, lif_chunks, store_bounds).  DMA chunks are large so each
    SBUF row transfer is long (small fp16 rows gate per-packet DMA rate);
    LIF chunks stay small for pipelining; stores drain at ~25-step bounds
    with a tiny final store for a short tail."""
    if t_steps == 100:
        # Input rides the Sync ring except chunk 2 (Scalar, in parallel with
        # chunk 1 — doubles early input bw before stores need that ring).
        # Striping MORE input chunks across both rings was A/B-tested and
        # loses ~4.5us: mid-run the queues just contend for the same
        # per-core HBM ceiling, while the finer early chunks stream slower.
        dma = [(0, 6, "sync"), (6, 20, "scalar"), (26, 26, "sync"),
               (52, 32, "sync"), (84, 16, "sync")]
        lif = [6, 10, 10, 13, 13, 16, 16, 10, 6]
        bounds = [16, 39, 52, 68, 84, 94, 100]
        # tail < t_steps (ship raw W fp16, host-decoded) was tried: the
        # doubled tail-store bytes flushed slower than the spike passes
        # they replaced (44.3us vs 38.9us).
        return dma, lif, bounds, t_steps
    dma, rem, s = [], t_steps, 0
    while rem > 0:
        c = min(24, rem)
        dma.append((s, c, "sync"))
        s += c
        rem -= c
    bounds = [s_ + c_ for s_, c_, _ in dma]
    return dma, list(c for _, c, _ in dma), bounds, t_steps


def _build_2x_uop(u1x):
    """2X_1PORT program for LIF_STEP_ANT, built by editing a copy of the
    REGULAR program.  Element0 (LO) computes on ALU stages 0-3 exactly like
    the 1x program; element1 (HI) enters through SRC_*_HI input lanes, rides
    the delay lanes, and computes on stages 4-7.  The LO result is captured
    into delay lane 0 at stage 4 and written back from there."""
    from concourse.dve_uop import AluInp, AluOp, DelayInp, InpSel, OutPath, OutSel

    u = deepcopy(u1x)
    u.inp = [InpSel.ZERO, InpSel.SRC_0, InpSel.CONST_0, InpSel.SRC_1,
             InpSel.CONST_1, InpSel.SRC_0_HI, InpSel.SRC_1_HI, InpSel.ZERO]
    u.inp_enable = [0, 1, 1, 1, 1, 1, 1, 0]

    HOLD = DelayInp.PREV_DELAY
    CAP = DelayInp.PREV_ALU_OUT
    s = u.datapath_config
    # delay lanes at stage 0: d0=W0 d1=d d2=x0 d3=th d4=W1 d5=x1

    def dp(i, op, a, b, lane_caps=()):
        st = s[i]
        st.op = op
        st.alu_src0 = a
        st.alu_src1 = b
        st.alu_out_enable = 1
        st.delay = [CAP if k in lane_caps else HOLD for k in range(7)]
        st.delay_enable = [1, 1, 1, 1, 1, 1, 0]

    dp(0, AluOp.MULTIPLY, AluInp.PREV_DELAY_0, AluInp.PREV_DELAY_1)  # W0*d
    dp(1, AluOp.ADD, AluInp.PREV_ALU_OUT, AluInp.PREV_DELAY_2)       # M0
    dp(2, AluOp.IS_GE, AluInp.PREV_DELAY_3, AluInp.PREV_ALU_OUT,
       lane_caps=(0,))                                               # mask0; d0<-M0
    dp(3, AluOp.MULTIPLY, AluInp.PREV_DELAY_0, AluInp.PREV_ALU_OUT)  # out0
    dp(4, AluOp.MULTIPLY, AluInp.PREV_DELAY_4, AluInp.PREV_DELAY_1,
       lane_caps=(0,))                                               # W1*d; d0<-out0
    dp(5, AluOp.ADD, AluInp.PREV_ALU_OUT, AluInp.PREV_DELAY_5)       # M1
    dp(6, AluOp.IS_GE, AluInp.PREV_DELAY_3, AluInp.PREV_ALU_OUT,
       lane_caps=(1,))                                               # mask1; d1<-M1
    dp(7, AluOp.MULTIPLY, AluInp.PREV_DELAY_1, AluInp.PREV_ALU_OUT)  # out1

    u.out = {OutPath.WR0_LO: OutSel.DELAY_0, OutPath.WR0_HI: OutSel.ALU_OUT,
             OutPath.WR1_LO: OutSel.ALU_OUT, OutPath.WR1_HI: OutSel.ALU_OUT}
    u.out_enable = {OutPath.WR0_LO: 1, OutPath.WR0_HI: 1,
                    OutPath.WR1_LO: 0, OutPath.WR1_HI: 0}
    return u


def _get_lif_op():
    """Register the fused LIF-step custom DVE op (idempotent)."""
    global _LIF_OP
    if _LIF_OP is not None:
        return _LIF_OP
    from concourse.dve_ops import (
        _COMPILE_CACHE, CUSTOM_DVE_SPECS, OPS, _SUB_OPCODE_FOR_NAME, DveOp,
    )
    from concourse.dve_spec import C0, C1, Spec, Src0, Src1, lower
    from concourse.dve_table_gen import dve_ver_for
    from concourse.dve_uop import DveOpSpec

    name = "LIF_STEP_ANT"
    if name in _SUB_OPCODE_FOR_NAME:
        _LIF_OP = next(op for op in OPS if op.name == name)
        return _LIF_OP

    M = Src0 * C0 + Src1

    def _ref(in0, in1, s0, s1, imm2):
        m = (in0.astype(np.float32) * np.float32(s0)
             + in1.astype(np.float32)).astype(np.float32)
        return np.where(m <= np.float32(s1), m, np.float32(0.0)).astype(np.float32)

    spec = Spec(body=M * (M <= C1), reference=_ref)
    row = max(_SUB_OPCODE_FOR_NAME.values()) + 1
    assert row < 0x20
    _SUB_OPCODE_FOR_NAME[name] = row

    assert dve_ver_for("TRN2") == "v3"
    uops_1x = lower(spec, ver="v3")
    assert len(uops_1x) == 1
    opspec = DveOpSpec(
        name=name, opcode=row, uops=uops_1x,
        uops_2x=[_build_2x_uop(uops_1x[0])] if LIF_2X else None,
        rd1_en=True, perf_max=1 if LIF_2X else 0,
    )
    shas = {"v3": opspec.sha("v3")}
    op = DveOp(name, spec, subdim=False, uops_sha=shas)
    OPS.append(op)
    CUSTOM_DVE_SPECS[name] = spec
    _COMPILE_CACHE[(name, "v3")] = opspec  # bypass lower() at table-gen/emit
    _LIF_OP = op
    return op


def _build_nc(t_steps: int, d_imm: float):
    import concourse.tile as tile
    from concourse import bacc, mybir

    w_dt = getattr(mybir.dt, W_DT)
    x_dt = getattr(mybir.dt, X_DT)

    lif_op = _get_lif_op()
    dma_chunks, lif_chunks, store_bounds, tail = _chunk_schedule(t_steps)
    assert sum(c for _, c, _ in dma_chunks) == t_steps
    assert sum(lif_chunks) == t_steps
    assert not store_bounds or store_bounds[-1] <= tail
    max_dc = max(c for _, c, _ in dma_chunks)

    nc = bacc.Bacc("TRN2", debug=False, target_bir_lowering=False)
    x_in = nc.dram_tensor("x", [P, t_steps * F], x_dt, kind="ExternalInput")
    s_out = nc.dram_tensor("s", [P, max(tail, 1) * F], mybir.dt.uint8,
                           kind="ExternalOutput")
    # steps >= tail ship as raw W (fp16); host decodes spike = (W == 0)
    w_out = (nc.dram_tensor("w", [P, (t_steps - tail) * F], w_dt,
                            kind="ExternalOutput")
             if tail < t_steps else None)

    with tile.TileContext(nc) as tcx, ExitStack() as ctx:
        xpool = ctx.enter_context(tcx.tile_pool(name="xp", bufs=4))
        spool = ctx.enter_context(tcx.tile_pool(name="sp", bufs=1))

        # Persistent state buffer: W[:, t*F:(t+1)*F] holds W_{t-1} (slot 0 is
        # the zero initial state, slot t+1 is W_t).
        wbuf = spool.tile([P, (t_steps + 1) * F], w_dt)
        nc.vector.memset(wbuf[:, 0:F], 0.0)
        # Persistent notspike-code buffer (u8; 0 <=> spike) for steps < tail.
        stbuf = spool.tile([P, max(tail, 1) * F], mybir.dt.uint8)

        def emit_spike(t0_, tc, mode="split"):
            # mode: "split" DVE/ACT by DVE_FRAC; "act" all-ACT (tail chunks,
            # overlaps the still-running LIF chain); "dve" all-DVE (the very
            # last chunk — shortest serial tail once the LIF chain is done).
            n = tc * F
            c_dve = {"split": (int(n * DVE_FRAC) // 16) * 16,
                     "act": 0, "dve": n}[mode]
            wsl = wbuf[:, (t0_ + 1) * F:(t0_ + tc + 1) * F]
            st = stbuf[:, t0_ * F:(t0_ + tc) * F]
            if c_dve > 0:
                nc.vector.tensor_scalar(
                    out=st[:, :c_dve], in0=wsl[:, :c_dve],
                    scalar1=0.0, scalar2=None,
                    op0=mybir.AluOpType.not_equal)
            if c_dve < n:
                nc.scalar.activation(
                    out=st[:, c_dve:n], in_=wsl[:, c_dve:n],
                    func=mybir.ActivationFunctionType.Abs, scale=1e19)

        def emit_store(s0_, s1_, eng):
            eng.dma_start(out=s_out[:, s0_ * F:s1_ * F],
                          in_=stbuf[:, s0_ * F:s1_ * F])

        # Input DMA chunks are wide (long SBUF rows sustain queue rate); LIF
        # consumes each loaded tile in smaller sub-chunks.  Spike passes trail
        # the LIF by one chunk; stores drain at store_bounds on the Scalar
        # ring, the last ones on the (by then input-free) Sync ring.
        dma_by_start = {s_: (c_, e_) for s_, c_, e_ in dma_chunks}
        xtiles = []  # (t0, dc, xt) of loaded chunks

        t0 = 0
        stored = 0
        bounds = list(store_bounds)
        prev = None
        pend = None  # contiguous [start, end) LIF-done but not yet spiked

        def flush_spikes(done, mode="split"):
            nonlocal pend, stored
            if pend is not None:
                emit_spike(pend[0], pend[1] - pend[0], mode=mode)
                pend = None
            while bounds and done >= bounds[0]:
                eng = nc.scalar if bounds[0] < 84 else nc.sync
                emit_store(stored, bounds[0], eng)
                stored = bounds.pop(0)

        for k, tc in enumerate(lif_chunks):
            if t0 in dma_by_start:
                dc, ename = dma_by_start[t0]
                xt = xpool.tile([P, max_dc * F], x_dt, tag="xt")
                eng = nc.sync if ename == "sync" else nc.scalar
                eng.dma_start(out=xt[:, :dc * F],
                              in_=x_in[:, t0 * F:(t0 + dc) * F])
                xtiles.append((t0, dc, xt))
            xt0, _, xt = next(e for e in xtiles if e[0] <= t0 < e[0] + e[1])
            xoff = t0 - xt0
            ins = nc.vector._custom_dve(
                lif_op,
                out=wbuf[:, (t0 + 1) * F:(t0 + tc + 1) * F],
                in0=wbuf[:, t0 * F:(t0 + tc) * F],
                in1=xt[:, xoff * F:(xoff + tc) * F],
                s0=d_imm, s1=THRESH)
            if LIF_2X:
                ins.ins.perf_max = 1  # byte-36[7:6] -> engine runs 2X_1PORT
            if prev is not None and prev[0] < tail:
                done = prev[0] + prev[1]
                if prev[0] >= 84:
                    # tail chunks: per-chunk, all-DVE (shortest serial tail)
                    emit_spike(*prev, mode="dve")
                    while bounds and done >= bounds[0]:
                        emit_store(stored, bounds[0], nc.sync)
                        stored = bounds.pop(0)
                else:
                    # Per-chunk spike slices.  (Merging slices across chunks
                    # to amortize instruction overhead was tried at several
                    # granularities: the long ACT slices delay store triggers
                    # queued behind them and always measured ~1-3us worse.)
                    pend = (prev[0] if pend is None else pend[0], done)
                    flush_spikes(done)
            prev = (t0, tc)
            t0 += tc
        if prev[0] < tail:
            flush_spikes(prev[0])
            emit_spike(*prev, mode="dve")
        for b in bounds:
            emit_store(stored, b, nc.sync)
            stored = b
        if w_out is not None:
            # Tail W stores: split across both rings so they flush in
            # parallel right after the last LIF chunk.
            mid = tail + (t_steps - tail) // 2
            nc.sync.dma_start(out=w_out[:, 0:(mid - tail) * F],
                              in_=wbuf[:, (tail + 1) * F:(mid + 1) * F])
            nc.scalar.dma_start(
                out=w_out[:, (mid - tail) * F:(t_steps - tail) * F],
                in_=wbuf[:, (mid + 1) * F:(t_steps + 1) * F])
    nc.compile()
    return nc


def _get_nc(t_steps: int, d_imm: float):
    key = (t_steps, np.float32(d_imm).tobytes(), LIF_2X, X_DT, W_DT)
    if key not in _BUILD_CACHE:
        _BUILD_CACHE[key] = _build_nc(t_steps, d_imm)
    return _BUILD_CACHE[key]


def _shard_x(x: np.ndarray) -> list[np.ndarray]:
    b, t, d = x.shape
    # [b, t, core, chunk, 256] -> [core, b, chunk, t, 256] -> [core, 128, t*256]
    xr = x.reshape(b, t, N_CORES, 4, F).transpose(2, 0, 3, 1, 4)
    xr = np.ascontiguousarray(xr).reshape(N_CORES, P, t * F)
    if X_FP16:
        xr = xr.astype(np.float16)
    return [xr[c] for c in range(N_CORES)]


def _unshard_spikes(codes: np.ndarray, t: int) -> np.ndarray:
    # codes: [core, 128, t*256] with 0 <=> spike (u8 notspike code for the
    # head steps, raw W fp16 bits for the tail steps — both decode the same).
    sr = (codes == 0).astype(np.float32).reshape(N_CORES, B, 4, t, F)
    sr = sr.transpose(1, 3, 0, 2, 4)
    return np.ascontiguousarray(sr).reshape(B, t, N_CORES * 4 * F)


def _sigmoid_f32(decay: np.ndarray) -> np.float32:
    import jax
    import jax.numpy as jnp
    d = np.asarray(jax.nn.sigmoid(jnp.asarray(decay, jnp.float32)))
    return np.float32(d.reshape(-1)[0])


def kernel(x: np.ndarray, decay: np.ndarray) -> np.ndarray:
    from concourse.bass_utils import run_bass_kernel_spmd

    x = np.asarray(x, dtype=np.float32)
    b, t, d = x.shape
    d_f32 = _sigmoid_f32(np.asarray(decay))

    nc = _get_nc(t, float(d_f32))
    shards = _shard_x(x)
    in_maps = [{"x": np.ascontiguousarray(s)} for s in shards]
    res = run_bass_kernel_spmd(nc, in_maps, core_ids=list(range(N_CORES)))
    parts = []
    for c in range(N_CORES):
        r = res.results[c]
        code = np.asarray(r["s"]) != 0
        if "w" in r:
            code = np.concatenate([code, np.asarray(r["w"]) != 0], axis=1)
        parts.append(code)
    return _unshard_spikes(np.stack(parts, axis=0), t)


# revision 66
# speedup vs baseline: 1.1685x; 1.0005x over previous
"""LIF spiking-neuron recurrence kernel for Trainium2 (8 NeuronCores, SPMD).

Problem: x [32, 100, 8192] f32, decay [1] f32.
    d = sigmoid(decay)
    mem_0 = x[:,0];  mem_t = mem_{t-1} * d * (1 - spike_{t-1}) + x[:,t]
    spike_t = (mem_t > 0.5);  out[:,t] = spike_t  (f32 0/1)

Device formulation:
    W_{-1} = 0
    M_t = (W_{t-1} * d) + x_t
    W_t = (M_t <= 0.5) * M_t
spike_t = (M_t > 0.5) = (W_t == 0) exactly. The whole step is ONE custom
DVE op over a persistent self-referential W buffer (out AP trails in0 by
exactly F elements):
    LIF_STEP_ANT: out = M * (M <= s1),  M = in0*s0 + in1

v2 changes over the 61.9us baseline (measured ~38.9-41us on the 8-core
axon TRN2 pod, ambient-dependent; bit-identical to the exact host
simulation of the fp16 pipeline):
  * x is cast to fp16 on the host: input DMA bytes halve (13.1 -> 6.55 MB
    per core).  W state is fp16 too.  Exact-simulated vs the f32 jax
    reference: 1860/26.2M spike flips = rel err 1.802e-2 (< 2e-2 gate).
  * hand-written 2X_1PORT uop program for the LIF op (all operands fp16,
    stride-1, 4B-aligned): the serial recurrence runs 2 elem/cycle/lane,
    27.7us -> 14us measured.  The REGULAR 1x program stays in the table;
    HW falls back to it silently if the pattern check fails (the
    instruction's perf_max byte-36[7:6] requests the mode).
  * spike conversion emits a u8 notspike code, split DVE (tensor_scalar
    not_equal, {0,1}) / ACT (single Abs(1e19*W) pass, saturates to
    {0,255}); host decodes spike = (code == 0).
  * input loads on the Sync-engine HWDGE ring (fires ~1.5us earlier in
    the preamble than Scalar's), with DMA chunk 2 loaded over the Scalar
    ring in parallel; u8 stores drain on Scalar mid-run, Sync at the tail.

Sharding: 32*8192 = 262144 independent (b, d) lanes split 8 ways by
feature blocks; per-core layout [128 partitions, T*256] with partition
p = b*4 + (d_local//256), free offset = t*256 + d_local%256.
"""

from contextlib import ExitStack
from copy import deepcopy

import numpy as np

N_CORES = 8
B, T, D = 32, 100, 8192
P = 128          # SBUF partitions
F = 256          # free elements per timestep per core (32*1024/128)
THRESH = 0.5

# --- tuning knobs ---------------------------------------------------------
LIF_2X = True           # register + request the 2X_1PORT uop program
X_FP16 = True           # cast x to fp16 on host (halves input DMA)
# Spike conversion writes a u8 "notspike" code: DVE tensor_scalar is_not_equal
# gives (W!=0) in {0,1}; ACT in ONE pass writes u8(Abs(1e19*W)) which
# saturates to {0,255}.  Both decode on host as spike = (code == 0).
# (Shipping W itself (fp16) was tried: the doubled output DMA starved the
# input stream and lost 7us net.  GpSimd tensor_scalar is ~16x below
# roofline AND starves concurrent DVE SBUF access — never give it compute.)
DVE_FRAC = 0.35         # fraction of spike columns on DVE (rest on ACT)

_BUILD_CACHE: dict = {}
_LIF_OP = None

W_DT = "float16" if LIF_2X or X_FP16 else "float32"
X_DT = "float16" if X_FP16 else "float32"


def _chunk_schedule(t_steps: int):
    """(dma_chunks, lif_chunks, store_bounds).  DMA chunks are large so each
    SBUF row transfer is long (small fp16 rows gate per-packet DMA rate);
    LIF chunks stay small for pipelining; stores drain at ~25-step bounds
    with a tiny final store for a short tail."""
    if t_steps == 100:
        # Input rides the Sync ring except chunk 2 (Scalar, in parallel with
        # chunk 1 — doubles early input bw before stores need that ring).
        # Striping MORE input chunks across both rings was A/B-tested and
        # loses ~4.5us: mid-run the queues just contend for the same
        # per-core HBM ceiling, while the finer early chunks stream slower.
        dma = [(0, 6, "sync"), (6, 20, "scalar"), (26, 26, "sync"),
               (52, 32, "sync"), (84, 16, "sync")]
        lif = [6, 10, 10, 13, 13, 16, 16, 10, 6]
        import os
        if os.environ.get("LIF_SCHED") == "B":
            # coalesced: fewer ACT-ring triggers mid-run
            bounds = [(26, "scalar"), (52, "scalar"), (84, "scalar"),
                      (94, "sync"), (100, "sync")]
        else:
            bounds = [(16, "scalar"), (39, "scalar"), (52, "scalar"),
                      (68, "scalar"), (84, "sync"), (94, "sync"),
                      (100, "sync")]
        # tail < t_steps (ship raw W fp16, host-decoded) was tried: the
        # doubled tail-store bytes flushed slower than the spike passes
        # they replaced (44.3us vs 38.9us).
        return dma, lif, bounds, t_steps
    dma, rem, s = [], t_steps, 0
    while rem > 0:
        c = min(24, rem)
        dma.append((s, c, "sync"))
        s += c
        rem -= c
    bounds = [(s_ + c_, "scalar") for s_, c_, _ in dma]
    return dma, list(c for _, c, _ in dma), bounds, t_steps


def _build_2x_uop(u1x):
    """2X_1PORT program for LIF_STEP_ANT, built by editing a copy of the
    REGULAR program.  Element0 (LO) computes on ALU stages 0-3 exactly like
    the 1x program; element1 (HI) enters through SRC_*_HI input lanes, rides
    the delay lanes, and computes on stages 4-7.  The LO result is captured
    into delay lane 0 at stage 4 and written back from there."""
    from concourse.dve_uop import AluInp, AluOp, DelayInp, InpSel, OutPath, OutSel

    u = deepcopy(u1x)
    u.inp = [InpSel.ZERO, InpSel.SRC_0, InpSel.CONST_0, InpSel.SRC_1,
             InpSel.CONST_1, InpSel.SRC_0_HI, InpSel.SRC_1_HI, InpSel.ZERO]
    u.inp_enable = [0, 1, 1, 1, 1, 1, 1, 0]

    HOLD = DelayInp.PREV_DELAY
    CAP = DelayInp.PREV_ALU_OUT
    s = u.datapath_config
    # delay lanes at stage 0: d0=W0 d1=d d2=x0 d3=th d4=W1 d5=x1

    def dp(i, op, a, b, lane_caps=()):
        st = s[i]
        st.op = op
        st.alu_src0 = a
        st.alu_src1 = b
        st.alu_out_enable = 1
        st.delay = [CAP if k in lane_caps else HOLD for k in range(7)]
        st.delay_enable = [1, 1, 1, 1, 1, 1, 0]

    dp(0, AluOp.MULTIPLY, AluInp.PREV_DELAY_0, AluInp.PREV_DELAY_1)  # W0*d
    dp(1, AluOp.ADD, AluInp.PREV_ALU_OUT, AluInp.PREV_DELAY_2)       # M0
    dp(2, AluOp.IS_GE, AluInp.PREV_DELAY_3, AluInp.PREV_ALU_OUT,
       lane_caps=(0,))                                               # mask0; d0<-M0
    dp(3, AluOp.MULTIPLY, AluInp.PREV_DELAY_0, AluInp.PREV_ALU_OUT)  # out0
    dp(4, AluOp.MULTIPLY, AluInp.PREV_DELAY_4, AluInp.PREV_DELAY_1,
       lane_caps=(0,))                                               # W1*d; d0<-out0
    dp(5, AluOp.ADD, AluInp.PREV_ALU_OUT, AluInp.PREV_DELAY_5)       # M1
    dp(6, AluOp.IS_GE, AluInp.PREV_DELAY_3, AluInp.PREV_ALU_OUT,
       lane_caps=(1,))                                               # mask1; d1<-M1
    dp(7, AluOp.MULTIPLY, AluInp.PREV_DELAY_1, AluInp.PREV_ALU_OUT)  # out1

    u.out = {OutPath.WR0_LO: OutSel.DELAY_0, OutPath.WR0_HI: OutSel.ALU_OUT,
             OutPath.WR1_LO: OutSel.ALU_OUT, OutPath.WR1_HI: OutSel.ALU_OUT}
    u.out_enable = {OutPath.WR0_LO: 1, OutPath.WR0_HI: 1,
                    OutPath.WR1_LO: 0, OutPath.WR1_HI: 0}
    return u


def _get_lif_op():
    """Register the fused LIF-step custom DVE op (idempotent)."""
    global _LIF_OP
    if _LIF_OP is not None:
        return _LIF_OP
    from concourse.dve_ops import (
        _COMPILE_CACHE, CUSTOM_DVE_SPECS, OPS, _SUB_OPCODE_FOR_NAME, DveOp,
    )
    from concourse.dve_spec import C0, C1, Spec, Src0, Src1, lower
    from concourse.dve_table_gen import dve_ver_for
    from concourse.dve_uop import DveOpSpec

    name = "LIF_STEP_ANT"
    if name in _SUB_OPCODE_FOR_NAME:
        _LIF_OP = next(op for op in OPS if op.name == name)
        return _LIF_OP

    M = Src0 * C0 + Src1

    def _ref(in0, in1, s0, s1, imm2):
        m = (in0.astype(np.float32) * np.float32(s0)
             + in1.astype(np.float32)).astype(np.float32)
        return np.where(m <= np.float32(s1), m, np.float32(0.0)).astype(np.float32)

    spec = Spec(body=M * (M <= C1), reference=_ref)
    row = max(_SUB_OPCODE_FOR_NAME.values()) + 1
    assert row < 0x20
    _SUB_OPCODE_FOR_NAME[name] = row

    assert dve_ver_for("TRN2") == "v3"
    uops_1x = lower(spec, ver="v3")
    assert len(uops_1x) == 1
    opspec = DveOpSpec(
        name=name, opcode=row, uops=uops_1x,
        uops_2x=[_build_2x_uop(uops_1x[0])] if LIF_2X else None,
        rd1_en=True, perf_max=1 if LIF_2X else 0,
    )
    shas = {"v3": opspec.sha("v3")}
    op = DveOp(name, spec, subdim=False, uops_sha=shas)
    OPS.append(op)
    CUSTOM_DVE_SPECS[name] = spec
    _COMPILE_CACHE[(name, "v3")] = opspec  # bypass lower() at table-gen/emit
    _LIF_OP = op
    return op


def _build_nc(t_steps: int, d_imm: float):
    import concourse.tile as tile
    from concourse import bacc, mybir

    w_dt = getattr(mybir.dt, W_DT)
    x_dt = getattr(mybir.dt, X_DT)

    lif_op = _get_lif_op()
    dma_chunks, lif_chunks, store_bounds, tail = _chunk_schedule(t_steps)
    assert sum(c for _, c, _ in dma_chunks) == t_steps
    assert sum(lif_chunks) == t_steps
    assert not store_bounds or store_bounds[-1][0] <= tail
    max_dc = max(c for _, c, _ in dma_chunks)

    nc = bacc.Bacc("TRN2", debug=False, target_bir_lowering=False)
    x_in = nc.dram_tensor("x", [P, t_steps * F], x_dt, kind="ExternalInput")
    s_out = nc.dram_tensor("s", [P, max(tail, 1) * F], mybir.dt.uint8,
                           kind="ExternalOutput")
    # steps >= tail ship as raw W (fp16); host decodes spike = (W == 0)
    w_out = (nc.dram_tensor("w", [P, (t_steps - tail) * F], w_dt,
                            kind="ExternalOutput")
             if tail < t_steps else None)

    with tile.TileContext(nc) as tcx, ExitStack() as ctx:
        xpool = ctx.enter_context(tcx.tile_pool(name="xp", bufs=4))
        spool = ctx.enter_context(tcx.tile_pool(name="sp", bufs=1))

        # Persistent state buffer: W[:, t*F:(t+1)*F] holds W_{t-1} (slot 0 is
        # the zero initial state, slot t+1 is W_t).
        wbuf = spool.tile([P, (t_steps + 1) * F], w_dt)
        nc.vector.memset(wbuf[:, 0:F], 0.0)
        # Persistent notspike-code buffer (u8; 0 <=> spike) for steps < tail.
        stbuf = spool.tile([P, max(tail, 1) * F], mybir.dt.uint8)

        def emit_spike(t0_, tc, mode="split"):
            # mode: "split" DVE/ACT by DVE_FRAC; "act" all-ACT (tail chunks,
            # overlaps the still-running LIF chain); "dve" all-DVE (the very
            # last chunk — shortest serial tail once the LIF chain is done).
            n = tc * F
            c_dve = {"split": (int(n * DVE_FRAC) // 16) * 16,
                     "act": 0, "dve": n}[mode]
            wsl = wbuf[:, (t0_ + 1) * F:(t0_ + tc + 1) * F]
            st = stbuf[:, t0_ * F:(t0_ + tc) * F]
            if c_dve > 0:
                nc.vector.tensor_scalar(
                    out=st[:, :c_dve], in0=wsl[:, :c_dve],
                    scalar1=0.0, scalar2=None,
                    op0=mybir.AluOpType.not_equal)
            if c_dve < n:
                nc.scalar.activation(
                    out=st[:, c_dve:n], in_=wsl[:, c_dve:n],
                    func=mybir.ActivationFunctionType.Abs, scale=1e19)

        def emit_store(s0_, s1_, eng):
            eng.dma_start(out=s_out[:, s0_ * F:s1_ * F],
                          in_=stbuf[:, s0_ * F:s1_ * F])

        # Input DMA chunks are wide (long SBUF rows sustain queue rate); LIF
        # consumes each loaded tile in smaller sub-chunks.  Spike passes trail
        # the LIF by one chunk; stores drain at store_bounds on the Scalar
        # ring, the last ones on the (by then input-free) Sync ring.
        dma_by_start = {s_: (c_, e_) for s_, c_, e_ in dma_chunks}
        xtiles = []  # (t0, dc, xt) of loaded chunks

        t0 = 0
        stored = 0
        bounds = list(store_bounds)
        prev = None
        pend = None  # contiguous [start, end) LIF-done but not yet spiked

        def flush_spikes(done, mode="split"):
            nonlocal pend, stored
            if pend is not None:
                emit_spike(pend[0], pend[1] - pend[0], mode=mode)
                pend = None
            while bounds and done >= bounds[0][0]:
                b, ename = bounds.pop(0)
                emit_store(stored, b, nc.sync if ename == "sync" else nc.scalar)
                stored = b

        for k, tc in enumerate(lif_chunks):
            if t0 in dma_by_start:
                dc, ename = dma_by_start[t0]
                xt = xpool.tile([P, max_dc * F], x_dt, tag="xt")
                eng = nc.sync if ename == "sync" else nc.scalar
                eng.dma_start(out=xt[:, :dc * F],
                              in_=x_in[:, t0 * F:(t0 + dc) * F])
                xtiles.append((t0, dc, xt))
            xt0, _, xt = next(e for e in xtiles if e[0] <= t0 < e[0] + e[1])
            xoff = t0 - xt0
            ins = nc.vector._custom_dve(
                lif_op,
                out=wbuf[:, (t0 + 1) * F:(t0 + tc + 1) * F],
                in0=wbuf[:, t0 * F:(t0 + tc) * F],
                in1=xt[:, xoff * F:(xoff + tc) * F],
                s0=d_imm, s1=THRESH)
            if LIF_2X:
                ins.ins.perf_max = 1  # byte-36[7:6] -> engine runs 2X_1PORT
            if prev is not None and prev[0] < tail:
                done = prev[0] + prev[1]
                if prev[0] >= 84:
                    # tail chunks: per-chunk, all-DVE (shortest serial tail)
                    emit_spike(*prev, mode="dve")
                    while bounds and done >= bounds[0][0]:
                        b, _ = bounds.pop(0)
                        emit_store(stored, b, nc.sync)
                        stored = b
                else:
                    # Per-chunk spike slices.  (Merging slices across chunks
                    # to amortize instruction overhead was tried at several
                    # granularities: the long ACT slices delay store triggers
                    # queued behind them and always measured ~1-3us worse.)
                    pend = (prev[0] if pend is None else pend[0], done)
                    flush_spikes(done)
            prev = (t0, tc)
            t0 += tc
        if prev[0] < tail:
            flush_spikes(prev[0])
            emit_spike(*prev, mode="dve")
        for b, _ in bounds:
            emit_store(stored, b, nc.sync)
            stored = b
        if w_out is not None:
            # Tail W stores: split across both rings so they flush in
            # parallel right after the last LIF chunk.
            mid = tail + (t_steps - tail) // 2
            nc.sync.dma_start(out=w_out[:, 0:(mid - tail) * F],
                              in_=wbuf[:, (tail + 1) * F:(mid + 1) * F])
            nc.scalar.dma_start(
                out=w_out[:, (mid - tail) * F:(t_steps - tail) * F],
                in_=wbuf[:, (mid + 1) * F:(t_steps + 1) * F])
    nc.compile()
    return nc


def _get_nc(t_steps: int, d_imm: float):
    key = (t_steps, np.float32(d_imm).tobytes(), LIF_2X, X_DT, W_DT)
    if key not in _BUILD_CACHE:
        _BUILD_CACHE[key] = _build_nc(t_steps, d_imm)
    return _BUILD_CACHE[key]


def _shard_x(x: np.ndarray) -> list[np.ndarray]:
    b, t, d = x.shape
    # [b, t, core, chunk, 256] -> [core, b, chunk, t, 256] -> [core, 128, t*256]
    xr = x.reshape(b, t, N_CORES, 4, F).transpose(2, 0, 3, 1, 4)
    xr = np.ascontiguousarray(xr).reshape(N_CORES, P, t * F)
    if X_FP16:
        xr = xr.astype(np.float16)
    return [xr[c] for c in range(N_CORES)]


def _unshard_spikes(codes: np.ndarray, t: int) -> np.ndarray:
    # codes: [core, 128, t*256] with 0 <=> spike (u8 notspike code for the
    # head steps, raw W fp16 bits for the tail steps — both decode the same).
    sr = (codes == 0).astype(np.float32).reshape(N_CORES, B, 4, t, F)
    sr = sr.transpose(1, 3, 0, 2, 4)
    return np.ascontiguousarray(sr).reshape(B, t, N_CORES * 4 * F)


def _sigmoid_f32(decay: np.ndarray) -> np.float32:
    import jax
    import jax.numpy as jnp
    d = np.asarray(jax.nn.sigmoid(jnp.asarray(decay, jnp.float32)))
    return np.float32(d.reshape(-1)[0])


def kernel(x: np.ndarray, decay: np.ndarray) -> np.ndarray:
    from concourse.bass_utils import run_bass_kernel_spmd

    x = np.asarray(x, dtype=np.float32)
    b, t, d = x.shape
    d_f32 = _sigmoid_f32(np.asarray(decay))

    nc = _get_nc(t, float(d_f32))
    shards = _shard_x(x)
    in_maps = [{"x": np.ascontiguousarray(s)} for s in shards]
    res = run_bass_kernel_spmd(nc, in_maps, core_ids=list(range(N_CORES)))
    parts = []
    for c in range(N_CORES):
        r = res.results[c]
        code = np.asarray(r["s"]) != 0
        if "w" in r:
            code = np.concatenate([code, np.asarray(r["w"]) != 0], axis=1)
        parts.append(code)
    return _unshard_spikes(np.stack(parts, axis=0), t)
